# revision 5
# baseline (speedup 1.0000x reference)
"""Trainium2 Bass kernel for batched 64-point DCT (flattened-patch GEMM).

Reference computation: out = x.reshape(b, -1, 64) @ K, reshaped back.
Pure data parallel over 8 NeuronCores: core i handles batch i as a
[49152, 64] x [64, 64] GEMM.

The problem is HBM-bandwidth bound (per core: 12 MiB in + 12 MiB out in
fp32).  The 2e-2 relative-error budget admits bf16 for both the input
stream and the output stream (measured rel err 4.2e-3), halving HBM
traffic to ~12.6 MB/core against the ~360 GB/s per-core DMA roofline.

Device layout (host-prepared, all bf16):
  x[(z*64+s), pair] = inp[2*pair+z, s]   -- [128, 24576]
  kblk = blockdiag(K, K)                 -- [128, 128], stationary
  y[(z*64+f), pair] = out[2*pair+z, f]   -- [128, 24576]

Matmul: kblk.T @ x_chunk -> PSUM [128, 512] fp32 (one bank per chunk),
then PSUM->SBUF bf16 copies alternate DVE/ACT, and tiles stream back out.

Scheduling: tiles are small at the ends (fast pipeline fill, short
drain) and 2048 pairs in the middle.  Each hwdge queue tops out around
~230 GB/s, so input and output are spread across the SP and ACT hardware
queues plus the gpsimd software-DGE queue (~140 GB/s) to lift aggregate
throughput above a single queue's ceiling.
"""

import numpy as np
import ml_dtypes

import concourse.mybir as mybir
from concourse import bacc
from concourse.bass_utils import run_bass_kernel_spmd
from concourse.tile import TileContext

P = 128       # SBUF partitions / blockdiag contraction dim
S = 64        # DCT size
N_CORES = 8
MAXTILE = 2048             # pair-columns per full tile
CHUNK = 512                # pair-columns per matmul (one PSUM bank)
BF16 = mybir.dt.bfloat16

# tile sizes in pair-columns: small at the ends, 2048 in the middle
TILE_SIZES = [512, 512, 1024] + [2048] * 10 + [1024, 512, 512]
# queue per input tile: 0=SP(sync), 1=ACT(scalar), 2=SWDGE(gpsimd)
IN_Q = [0] * len(TILE_SIZES)
# queue per output tile (2 and 1 alternating, as in v2)
OUT_Q = [2 if i % 2 == 0 else 1 for i in range(len(TILE_SIZES))]


def build_kernel(n_patches: int):
    n_pairs = n_patches // 2
    assert sum(TILE_SIZES) == n_pairs
    nc = bacc.Bacc(
        "TRN2",
        target_bir_lowering=False,
        debug=False,
        enable_asserts=False,
        num_devices=N_CORES,
    )
    x = nc.dram_tensor("x", [P, n_pairs], BF16, kind="ExternalInput")
    k = nc.dram_tensor("k", [P, P], BF16, kind="ExternalInput")
    y = nc.dram_tensor("y", [P, n_pairs], BF16, kind="ExternalOutput")

    xap = x.ap()
    yap = y.ap()

    with TileContext(nc) as tc:
        with (
            tc.tile_pool(name="consts", bufs=1) as consts,
            tc.tile_pool(name="xin", bufs=6) as x_pool,
            tc.tile_pool(name="outsb", bufs=6) as out_pool,
            tc.tile_pool(name="pout", bufs=8, space="PSUM") as pout_pool,
        ):
            queues = [nc.sync, nc.scalar, nc.gpsimd]
            kblk = consts.tile([P, P], BF16)
            nc.scalar.dma_start(out=kblk[:], in_=k.ap())

            col = 0
            nchunk = 0
            for ti, size in enumerate(TILE_SIZES):
                x_tile = x_pool.tile(
                    [P, MAXTILE], BF16, tag="x_tile", name=f"x_t{ti}"
                )
                queues[IN_Q[ti]].dma_start(
                    out=x_tile[:, :size], in_=xap[:, col : col + size]
                )
                out_sb = out_pool.tile(
                    [P, MAXTILE], BF16, tag="out_sb", name=f"y_t{ti}"
                )
                for j in range(size // CHUNK):
                    po = pout_pool.tile([P, CHUNK], mybir.dt.float32)
                    nc.tensor.matmul(
                        po[:],
                        lhsT=kblk[:],
                        rhs=x_tile[:, CHUNK * j : CHUNK * (j + 1)],
                        start=True,
                        stop=True,
                    )
                    # gpsimd cannot read PSUM; split copies over DVE/ACT
                    if nchunk % 2 == 0:
                        nc.vector.tensor_copy(
                            out_sb[:, CHUNK * j : CHUNK * (j + 1)], po[:]
                        )
                    else:
                        nc.scalar.copy(
                            out_sb[:, CHUNK * j : CHUNK * (j + 1)], po[:]
                        )
                    nchunk += 1
                queues[OUT_Q[ti]].dma_start(
                    out=yap[:, col : col + size], in_=out_sb[:, :size]
                )
                col += size
    nc.compile()
    return nc


def prep_inputs(x_full: np.ndarray, kmat: np.ndarray):
    """Full [8, C, H, W] fp32 -> per-core device in_maps (bf16)."""
    b = x_full.shape[0]
    n_patches = x_full[0].size // S
    n_pairs = n_patches // 2
    x16 = x_full.astype(ml_dtypes.bfloat16)
    # [b, n_pairs, 2, 64] -> [b, (z s), pair]
    xt = np.ascontiguousarray(
        x16.reshape(b, n_pairs, 2 * S).transpose(0, 2, 1)
    ).reshape(b, P, n_pairs)
    kblk = np.zeros((P, P), dtype=ml_dtypes.bfloat16)
    kblk[:S, :S] = kmat.astype(ml_dtypes.bfloat16)
    kblk[S:, S:] = kmat.astype(ml_dtypes.bfloat16)
    return [{"x": xt[i], "k": kblk} for i in range(b)]


def unshard_output(res_results, shape):
    """Per-core y [128, n_pairs] bf16 -> full fp32 [8, C, H, W]."""
    b, c, h, w = shape
    n_pairs = c * h * w // S // 2
    outs = []
    for i in range(b):
        yv = np.asarray(res_results[i]["y"]).reshape(2, S, n_pairs)
        # out[2p+z, f] = y[(z f), p]
        o = yv.transpose(2, 0, 1).astype(np.float32).reshape(c, h, w)
        outs.append(o)
    return np.stack(outs, axis=0)


def kernel(inputs, kernel):
    x_full = np.asarray(inputs, dtype=np.float32)
    kmat = np.asarray(kernel, dtype=np.float32)
    b, c, h, w = x_full.shape
    assert b == N_CORES, f"expected batch {N_CORES}, got {b}"
    n_patches = c * h * w // S
    nc = build_kernel(n_patches)
    in_maps = prep_inputs(x_full, kmat)
    res = run_bass_kernel_spmd(nc, in_maps, core_ids=list(range(N_CORES)))
    return unshard_output(res.results, (b, c, h, w))
